# revision 5
# baseline (speedup 1.0000x reference)
"""Trainium2 kernel for nn_BFM_torch_56384330662315 (gnn_message_passing).

Reference semantics (B=4, C=128, N=2048, K=16):
  feats = transpose(seg_features, (0,2,1))                 # [B,N,C]
  per sample: adj = boundary-cut symmetric kNN graph; two GCN layers
  out = refined + feats

Each GCN layer computes ``out = (adj/deg) @ f + feat`` but returns plain
``feat`` whenever any node has zero degree (``has_zero`` in the reference).
Any node classified as a boundary node (argmax(edge_preds)==1) gets its row
AND column zeroed in the symmetric adjacency, so it has zero degree.  Hence
for every sample that has at least one edge node and at least one non-edge
node, both GCN layers are exact identities and the whole module reduces,
bit-for-bit in fp32, to:

  out = 2 * transpose(seg_features, (0,2,1))

The device kernel evaluates that scaled transpose, data-parallel over the
8 NeuronCores.  Each core handles a [128, 1024] half-sample in bfloat16
(rel err ~3e-3, well under the 2e-2 gate), raw bass with manual
semaphores (TileContext's epilogue barrier costs ~500ns of tail):

  - two DMA-transpose (XBAR) loads bring x^T into SBUF already transposed
    ([4,4] blocks: the second transfer starts right as the first ends,
    given HWDGE descriptor generations serialize at ~625ns);
  - DVE applies the x2 in two chunks (the only arithmetic left);
  - two SP HWDGE stores write the result via 2KB/partition descriptors
    into a packed [128, 8*128] DRAM layout (the host unpermutes, a pure
    reshape), their descriptor generations pipelined behind the scales.

(Prepared/triggered SWDGE stores would cut another ~1.2us of store-issue
latency and sim fine, but crash real silicon - NRT_EXEC_UNIT_UNRECOVERABLE;
the deferred-read ordering the simulators honor is not enforced by the
hardware.  Confirmed broken in two separate sessions; do not retry.)

The per-sample condition is checked on host from edge_preds (tiny);
samples that don't satisfy it (probability ~2^-2047 for the randn inputs
this problem is generated with) fall back to an exact numpy port of the
reference.
"""

import os

import numpy as np

# recover cleanly if a previous kernel left a NeuronCore exec unit wedged
os.environ.setdefault("NEURON_RT_RESET_CORES", "1")

B, C, N, K = 4, 128, 2048, 16
GEO_FILL = 1000.0
NCORES = 8
NSH = N * B // NCORES  # 1024 columns of seg_features per core
NBLK = NSH // 128      # 8 transpose blocks per core

# per-core program shape (sweep-tuned in TimelineSim)
SPLIT = [4, 4]

_EXEC = None  # cached (jitted shard_map, metadata)


def _bf16():
    import ml_dtypes

    return np.dtype(ml_dtypes.bfloat16)


def _build_nc():
    """Per-core program: y[128, 8*128] = packed 2*x[128,1024]^T in bf16.

    y[p, i*128 + c] = 2 * x[c, i*128 + p]  (host unpermutes blocks).

    Raw bass (no TileContext — its epilogue barrier costs ~500ns on the
    tail); manual semaphores order the three stages.  Pipeline
    (TimelineSim-tuned, [4,4] block splits): two XBAR dma-transpose loads
    (HWDGE descriptor generations serialize at ~625ns, so the second
    transfer starts right as the first ends), two DVE x2 scales (DVE is 3x
    faster than ACT here), two SP HWDGE stores whose descriptor
    generations pipeline exactly behind the scales.
    """
    from concourse import bacc, mybir

    BF16 = mybir.dt.bfloat16
    nc = bacc.Bacc(
        "TRN2",
        target_bir_lowering=False,
        debug=False,
        num_devices=NCORES,
    )
    x = nc.dram_tensor("x", [C, NSH], BF16, kind="ExternalInput").ap()
    y = nc.dram_tensor("y", [128, NSH], BF16, kind="ExternalOutput").ap()

    xt = nc.alloc_sbuf_tensor("xt_raw", [128, NBLK, 128], BF16).ap()
    ot = nc.alloc_sbuf_tensor("ot_raw", [128, NBLK, 128], BF16).ap()
    x_sem = nc.alloc_semaphore("x_sem")
    s_sem = nc.alloc_semaphore("s_sem")
    y_sem = nc.alloc_semaphore("y_sem")

    b0 = 0
    for nb in SPLIT:
        nc.sync.dma_start(
            xt[:, b0 : b0 + nb, :],
            x[:, b0 * 128 : (b0 + nb) * 128],
            transpose=True,
        ).then_inc(x_sem, 16)
        b0 += nb

    b0 = 0
    for j, nb in enumerate(SPLIT):
        nc.vector.wait_ge(x_sem, 16 * (j + 1))
        nc.vector.tensor_scalar_mul(
            ot[:, b0 : b0 + nb, :], xt[:, b0 : b0 + nb, :], 2.0
        ).then_inc(s_sem, 1)
        b0 += nb

    b0 = 0
    for j, nb in enumerate(SPLIT):
        nc.sync.wait_ge(s_sem, j + 1)
        nc.sync.dma_start(
            y[:, b0 * 128 : (b0 + nb) * 128],
            ot[:, b0 : b0 + nb, :],
        ).then_inc(y_sem, 16)
        b0 += nb

    nc.sync.wait_ge(y_sem, 16 * len(SPLIT))
    nc.compile()
    return nc


def _get_exec():
    """Build the per-core Bass program once and wrap it in a cached
    jit(shard_map) over the 8 cores (mirrors bass2jax.run_bass_via_pjrt)."""
    global _EXEC
    if _EXEC is not None:
        return _EXEC

    import jax
    from jax.experimental.shard_map import shard_map
    from jax.sharding import Mesh, PartitionSpec

    from concourse import bass2jax, mybir

    bass2jax.install_neuronx_cc_hook()
    nc = _build_nc()
    partition_name = nc.partition_id_tensor.name if nc.partition_id_tensor else None

    in_names: list = []
    out_names: list = []
    out_avals: list = []
    zero_outs: list = []
    for alloc in nc.m.functions[0].allocations:
        if not isinstance(alloc, mybir.MemoryLocationSet):
            continue
        name = alloc.memorylocations[0].name
        if alloc.kind == "ExternalInput":
            if name != partition_name:
                in_names.append(name)
        elif alloc.kind == "ExternalOutput":
            out_names.append(name)
            shape = tuple(alloc.tensor_shape)
            dtype = mybir.dt.np(alloc.dtype)
            out_avals.append(jax.core.ShapedArray(shape, dtype))
            zero_outs.append(np.zeros(shape, dtype))
    n_params = len(in_names)
    n_outs = len(out_avals)
    all_names = in_names + out_names
    if partition_name is not None:
        all_names.append(partition_name)

    def _body(*args):
        operands = list(args)
        if partition_name is not None:
            operands.append(bass2jax.partition_id_tensor())
        outs = bass2jax._bass_exec_p.bind(
            *operands,
            out_avals=tuple(out_avals),
            in_names=tuple(all_names),
            out_names=tuple(out_names),
            lowering_input_output_aliases=(),
            sim_require_finite=True,
            sim_require_nnan=True,
            nc=nc,
        )
        return tuple(outs)

    devices = jax.devices()[:NCORES]
    assert len(devices) == NCORES, f"need {NCORES} cores, have {len(jax.devices())}"
    mesh = Mesh(np.asarray(devices), ("core",))
    in_specs = (PartitionSpec("core"),) * (n_params + n_outs)
    out_specs = (PartitionSpec("core"),) * n_outs
    donate = tuple(range(n_params, n_params + n_outs))
    sharded = jax.jit(
        shard_map(
            _body, mesh=mesh, in_specs=in_specs, out_specs=out_specs, check_rep=False
        ),
        donate_argnums=donate,
        keep_unused=True,
    )
    _EXEC = (sharded, in_names, out_names, out_avals, zero_outs)
    return _EXEC


def _make_concat_inputs(seg: np.ndarray):
    """Per-core bf16 input shards, concatenated on axis 0 for shard_map."""
    bf16 = _bf16()
    xs = []
    for k in range(NCORES):
        b, h = k // 2, k % 2
        xs.append(seg[b, :, h * NSH : (h + 1) * NSH].astype(bf16))
    return {"x": np.concatenate(xs, axis=0)}


def _run_device(seg: np.ndarray) -> np.ndarray:
    """seg [B,C,N] f32 -> 2*transpose [B,N,C] f32 on the 8 cores (bf16
    internally, rel err ~1e-3), with retry and a host fallback in case a
    previous session left the accelerator wedged."""
    last_err = None
    for attempt in range(2):
        try:
            return _run_device_once(seg)
        except Exception as e:  # transient NRT_EXEC_UNIT_UNRECOVERABLE etc.
            last_err = e
    import sys

    print(
        f"kernel: device path failed twice ({type(last_err).__name__}: "
        f"{last_err}); computing on host",
        file=sys.stderr,
    )
    return np.ascontiguousarray(2.0 * seg.transpose(0, 2, 1))


def _run_device_once(seg: np.ndarray) -> np.ndarray:
    sharded, in_names, out_names, out_avals, zero_outs = _get_exec()
    by_name = _make_concat_inputs(seg)
    concat_in = [by_name[n] for n in in_names]
    concat_zeros = [
        np.zeros((NCORES * z.shape[0], *z.shape[1:]), z.dtype) for z in zero_outs
    ]
    out_arrs = sharded(*concat_in, *concat_zeros)
    y = np.asarray(out_arrs[out_names.index("y")]).reshape(NCORES, 128, NBLK, 128)

    out = np.empty((B, N, C), dtype=np.float32)
    for k in range(NCORES):
        b, h = k // 2, k % 2
        # y[k][p, i, c] = 2*x[c, i*128+p] -> rows i*128+p of the half-sample
        out[b, h * NSH : (h + 1) * NSH, :] = (
            y[k].transpose(1, 0, 2).reshape(NSH, C).astype(np.float32)
        )
    return out


# ---------------------------------------------------------------------------
# Exact numpy port of the reference — fallback for samples where the GCN does
# not collapse to identity (never hit for this problem's input distribution).
# ---------------------------------------------------------------------------


def _np_build_adj(g, edge_cls, k):
    n = g.shape[0]
    nbrs = np.argsort(g, axis=-1, kind="stable")[:, :k]
    rows = np.arange(n)[:, None]
    adj = np.zeros((n, n), g.dtype)
    adj[rows, nbrs] = 1.0
    adj[nbrs, rows] = 1.0
    is_edge = edge_cls == 1
    adj = np.where(is_edge[:, None], 0.0, adj)
    edge_col = is_edge[None, :]
    cond = (adj == 1) & edge_col
    maxgeo = np.min(np.where(cond, g, GEO_FILL), axis=-1)
    adjr = np.where(g > maxgeo[:, None], 0.0, adj)
    adjr = np.where(edge_col, 0.0, adjr)
    adj2 = np.where(is_edge[:, None], 0.0, adjr)
    adj_sym = ((adj2 > 0) | (adj2.T > 0)).astype(g.dtype)
    if np.all(is_edge):
        return np.eye(n, dtype=g.dtype)
    return adj_sym


def _np_gcn(feat, adj, W, b):
    identity = feat
    f = np.maximum(feat @ W.T + b, 0.0).astype(np.float32)
    row_deg = np.sum(adj, axis=-1, keepdims=True)
    col_deg = np.sum(adj, axis=-2, keepdims=True)
    degree = np.sqrt(row_deg) @ np.sqrt(col_deg)
    if np.any(degree == 0):
        return identity
    out = (adj / degree) @ f + identity
    return out.astype(np.float32)


def _np_sample(feat, ep, g, W1, b1, W2, b2):
    edge_cls = np.argmax(ep, axis=0)
    adj = _np_build_adj(g, edge_cls, K)
    r = _np_gcn(feat, adj, W1, b1)
    r = _np_gcn(r, adj, W2, b2)
    return r


def kernel(**inputs) -> np.ndarray:
    seg = np.ascontiguousarray(np.asarray(inputs["seg_features"], dtype=np.float32))
    ep = np.asarray(inputs["edge_preds"], dtype=np.float32)

    # argmax over the 2 class logits: class 1 iff ep[1] > ep[0] (ties -> 0)
    edge = ep[:, 1, :] > ep[:, 0, :]
    any_e = edge.any(axis=1)
    all_e = edge.all(axis=1)
    fast = any_e & ~all_e  # GCN layers are exact identities

    out = _run_device(seg)  # 2 * transpose, correct wherever fast[b]

    if not fast.all():
        g_all = np.asarray(inputs["gmatrix"], dtype=np.float32)
        W1 = np.asarray(inputs["W1"], dtype=np.float32)
        b1 = np.asarray(inputs["b1"], dtype=np.float32)
        W2 = np.asarray(inputs["W2"], dtype=np.float32)
        b2 = np.asarray(inputs["b2"], dtype=np.float32)
        for b in range(B):
            if not fast[b]:
                feat = np.ascontiguousarray(seg[b].T)
                r = _np_sample(feat, ep[b], g_all[b], W1, b1, W2, b2)
                out[b] = r + feat
    return out


# revision 6
# speedup vs baseline: 1.0573x; 1.0573x over previous
"""Trainium2 kernel for nn_BFM_torch_56384330662315 (gnn_message_passing).

Reference semantics (B=4, C=128, N=2048, K=16):
  feats = transpose(seg_features, (0,2,1))                 # [B,N,C]
  per sample: adj = boundary-cut symmetric kNN graph; two GCN layers
  out = refined + feats

Each GCN layer computes ``out = (adj/deg) @ f + feat`` but returns plain
``feat`` whenever any node has zero degree (``has_zero`` in the reference).
Any node classified as a boundary node (argmax(edge_preds)==1) gets its row
AND column zeroed in the symmetric adjacency, so it has zero degree.  Hence
for every sample that has at least one edge node and at least one non-edge
node, both GCN layers are exact identities and the whole module reduces,
bit-for-bit in fp32, to:

  out = 2 * transpose(seg_features, (0,2,1))

The device kernel evaluates that scaled transpose, data-parallel over the
8 NeuronCores.  Each core handles a [128, 1024] half-sample in bfloat16
(rel err ~3e-3, well under the 2e-2 gate), raw bass with manual
semaphores (TileContext's epilogue barrier costs ~500ns of tail):

  - two DMA-transpose (XBAR) loads bring x^T into SBUF already transposed
    ([4,4] blocks: the second transfer starts right as the first ends,
    given HWDGE descriptor generations serialize at ~625ns);
  - DVE applies the x2 in two chunks (the only arithmetic left);
  - two SP HWDGE stores write the result via 2KB/partition descriptors
    into a packed [128, 8*128] DRAM layout (the host unpermutes, a pure
    reshape), their descriptor generations pipelined behind the scales.

(Prepared/triggered SWDGE stores would cut another ~1.2us of store-issue
latency and sim fine, but crash real silicon - NRT_EXEC_UNIT_UNRECOVERABLE;
the deferred-read ordering the simulators honor is not enforced by the
hardware.  Confirmed broken in two separate sessions; do not retry.)

The per-sample condition is checked on host from edge_preds (tiny);
samples that don't satisfy it (probability ~2^-2047 for the randn inputs
this problem is generated with) fall back to an exact numpy port of the
reference.
"""

import os

import numpy as np

# recover cleanly if a previous kernel left a NeuronCore exec unit wedged
os.environ.setdefault("NEURON_RT_RESET_CORES", "1")

B, C, N, K = 4, 128, 2048, 16
GEO_FILL = 1000.0
NCORES = 8
NSH = N * B // NCORES  # 1024 columns of seg_features per core
NBLK = NSH // 128      # 8 transpose blocks per core

# per-core program shape (sweep-tuned in TimelineSim)
SPLIT = [4, 4]

_EXEC = None  # cached (jitted shard_map, metadata)


def _bf16():
    import ml_dtypes

    return np.dtype(ml_dtypes.bfloat16)


def _build_nc():
    """Per-core program: y[128, 8*128] = packed 2*x[128,1024]^T in bf16.

    y[p, i*128 + c] = 2 * x[c, i*128 + p]  (host unpermutes blocks).

    Raw bass (no TileContext — its epilogue barrier costs ~500ns on the
    tail); manual semaphores order the three stages.  Pipeline
    (TimelineSim-tuned, [4,4] block splits): two XBAR dma-transpose loads
    (HWDGE descriptor generations serialize at ~625ns, so the second
    transfer starts right as the first ends), two DVE x2 scales (DVE is 3x
    faster than ACT here), two SP HWDGE stores whose descriptor
    generations pipeline exactly behind the scales.
    """
    from concourse import bacc, mybir

    BF16 = mybir.dt.bfloat16
    nc = bacc.Bacc(
        "TRN2",
        target_bir_lowering=False,
        debug=False,
        num_devices=NCORES,
    )
    # Reorder the framework's 4 const-tile memsets to after the start
    # barrier: they serialize on Pool and gate the all-engine barrier
    # (~370ns), but nothing in this program reads the const tiles, so they
    # can run in Pool's idle window during the loads instead.  Same
    # instructions, same semantics for every consumer this program has.
    blk = nc.m.functions[0].blocks[0]
    insts = list(blk.instructions)
    memsets = [i for i in insts if isinstance(i, mybir.InstMemset)]
    assert len(memsets) == 4, [str(i) for i in memsets]
    blk.instructions = [
        i for i in insts if not isinstance(i, mybir.InstMemset)
    ] + memsets

    x = nc.dram_tensor("x", [C, NSH], BF16, kind="ExternalInput").ap()
    y = nc.dram_tensor("y", [128, NSH], BF16, kind="ExternalOutput").ap()

    xt = nc.alloc_sbuf_tensor("xt_raw", [128, NBLK, 128], BF16).ap()
    ot = nc.alloc_sbuf_tensor("ot_raw", [128, NBLK, 128], BF16).ap()
    x_sem = nc.alloc_semaphore("x_sem")
    s_sem = nc.alloc_semaphore("s_sem")
    y_sem = nc.alloc_semaphore("y_sem")

    b0 = 0
    for nb in SPLIT:
        nc.sync.dma_start(
            xt[:, b0 : b0 + nb, :],
            x[:, b0 * 128 : (b0 + nb) * 128],
            transpose=True,
        ).then_inc(x_sem, 16)
        b0 += nb

    b0 = 0
    for j, nb in enumerate(SPLIT):
        nc.vector.wait_ge(x_sem, 16 * (j + 1))
        nc.vector.tensor_scalar_mul(
            ot[:, b0 : b0 + nb, :], xt[:, b0 : b0 + nb, :], 2.0
        ).then_inc(s_sem, 1)
        b0 += nb

    b0 = 0
    for j, nb in enumerate(SPLIT):
        nc.sync.wait_ge(s_sem, j + 1)
        nc.sync.dma_start(
            y[:, b0 * 128 : (b0 + nb) * 128],
            ot[:, b0 : b0 + nb, :],
        ).then_inc(y_sem, 16)
        b0 += nb

    nc.sync.wait_ge(y_sem, 16 * len(SPLIT))
    nc.compile()
    return nc


def _get_exec():
    """Build the per-core Bass program once and wrap it in a cached
    jit(shard_map) over the 8 cores (mirrors bass2jax.run_bass_via_pjrt)."""
    global _EXEC
    if _EXEC is not None:
        return _EXEC

    import jax
    from jax.experimental.shard_map import shard_map
    from jax.sharding import Mesh, PartitionSpec

    from concourse import bass2jax, mybir

    bass2jax.install_neuronx_cc_hook()
    nc = _build_nc()
    partition_name = nc.partition_id_tensor.name if nc.partition_id_tensor else None

    in_names: list = []
    out_names: list = []
    out_avals: list = []
    zero_outs: list = []
    for alloc in nc.m.functions[0].allocations:
        if not isinstance(alloc, mybir.MemoryLocationSet):
            continue
        name = alloc.memorylocations[0].name
        if alloc.kind == "ExternalInput":
            if name != partition_name:
                in_names.append(name)
        elif alloc.kind == "ExternalOutput":
            out_names.append(name)
            shape = tuple(alloc.tensor_shape)
            dtype = mybir.dt.np(alloc.dtype)
            out_avals.append(jax.core.ShapedArray(shape, dtype))
            zero_outs.append(np.zeros(shape, dtype))
    n_params = len(in_names)
    n_outs = len(out_avals)
    all_names = in_names + out_names
    if partition_name is not None:
        all_names.append(partition_name)

    def _body(*args):
        operands = list(args)
        if partition_name is not None:
            operands.append(bass2jax.partition_id_tensor())
        outs = bass2jax._bass_exec_p.bind(
            *operands,
            out_avals=tuple(out_avals),
            in_names=tuple(all_names),
            out_names=tuple(out_names),
            lowering_input_output_aliases=(),
            sim_require_finite=True,
            sim_require_nnan=True,
            nc=nc,
        )
        return tuple(outs)

    devices = jax.devices()[:NCORES]
    assert len(devices) == NCORES, f"need {NCORES} cores, have {len(jax.devices())}"
    mesh = Mesh(np.asarray(devices), ("core",))
    in_specs = (PartitionSpec("core"),) * (n_params + n_outs)
    out_specs = (PartitionSpec("core"),) * n_outs
    donate = tuple(range(n_params, n_params + n_outs))
    sharded = jax.jit(
        shard_map(
            _body, mesh=mesh, in_specs=in_specs, out_specs=out_specs, check_rep=False
        ),
        donate_argnums=donate,
        keep_unused=True,
    )
    _EXEC = (sharded, in_names, out_names, out_avals, zero_outs)
    return _EXEC


def _make_concat_inputs(seg: np.ndarray):
    """Per-core bf16 input shards, concatenated on axis 0 for shard_map."""
    bf16 = _bf16()
    xs = []
    for k in range(NCORES):
        b, h = k // 2, k % 2
        xs.append(seg[b, :, h * NSH : (h + 1) * NSH].astype(bf16))
    return {"x": np.concatenate(xs, axis=0)}


def _run_device(seg: np.ndarray) -> np.ndarray:
    """seg [B,C,N] f32 -> 2*transpose [B,N,C] f32 on the 8 cores (bf16
    internally, rel err ~1e-3), with retry and a host fallback in case a
    previous session left the accelerator wedged."""
    last_err = None
    for attempt in range(2):
        try:
            return _run_device_once(seg)
        except Exception as e:  # transient NRT_EXEC_UNIT_UNRECOVERABLE etc.
            last_err = e
    import sys

    print(
        f"kernel: device path failed twice ({type(last_err).__name__}: "
        f"{last_err}); computing on host",
        file=sys.stderr,
    )
    return np.ascontiguousarray(2.0 * seg.transpose(0, 2, 1))


def _run_device_once(seg: np.ndarray) -> np.ndarray:
    sharded, in_names, out_names, out_avals, zero_outs = _get_exec()
    by_name = _make_concat_inputs(seg)
    concat_in = [by_name[n] for n in in_names]
    concat_zeros = [
        np.zeros((NCORES * z.shape[0], *z.shape[1:]), z.dtype) for z in zero_outs
    ]
    out_arrs = sharded(*concat_in, *concat_zeros)
    y = np.asarray(out_arrs[out_names.index("y")]).reshape(NCORES, 128, NBLK, 128)

    out = np.empty((B, N, C), dtype=np.float32)
    for k in range(NCORES):
        b, h = k // 2, k % 2
        # y[k][p, i, c] = 2*x[c, i*128+p] -> rows i*128+p of the half-sample
        out[b, h * NSH : (h + 1) * NSH, :] = (
            y[k].transpose(1, 0, 2).reshape(NSH, C).astype(np.float32)
        )
    return out


# ---------------------------------------------------------------------------
# Exact numpy port of the reference — fallback for samples where the GCN does
# not collapse to identity (never hit for this problem's input distribution).
# ---------------------------------------------------------------------------


def _np_build_adj(g, edge_cls, k):
    n = g.shape[0]
    nbrs = np.argsort(g, axis=-1, kind="stable")[:, :k]
    rows = np.arange(n)[:, None]
    adj = np.zeros((n, n), g.dtype)
    adj[rows, nbrs] = 1.0
    adj[nbrs, rows] = 1.0
    is_edge = edge_cls == 1
    adj = np.where(is_edge[:, None], 0.0, adj)
    edge_col = is_edge[None, :]
    cond = (adj == 1) & edge_col
    maxgeo = np.min(np.where(cond, g, GEO_FILL), axis=-1)
    adjr = np.where(g > maxgeo[:, None], 0.0, adj)
    adjr = np.where(edge_col, 0.0, adjr)
    adj2 = np.where(is_edge[:, None], 0.0, adjr)
    adj_sym = ((adj2 > 0) | (adj2.T > 0)).astype(g.dtype)
    if np.all(is_edge):
        return np.eye(n, dtype=g.dtype)
    return adj_sym


def _np_gcn(feat, adj, W, b):
    identity = feat
    f = np.maximum(feat @ W.T + b, 0.0).astype(np.float32)
    row_deg = np.sum(adj, axis=-1, keepdims=True)
    col_deg = np.sum(adj, axis=-2, keepdims=True)
    degree = np.sqrt(row_deg) @ np.sqrt(col_deg)
    if np.any(degree == 0):
        return identity
    out = (adj / degree) @ f + identity
    return out.astype(np.float32)


def _np_sample(feat, ep, g, W1, b1, W2, b2):
    edge_cls = np.argmax(ep, axis=0)
    adj = _np_build_adj(g, edge_cls, K)
    r = _np_gcn(feat, adj, W1, b1)
    r = _np_gcn(r, adj, W2, b2)
    return r


def kernel(**inputs) -> np.ndarray:
    seg = np.ascontiguousarray(np.asarray(inputs["seg_features"], dtype=np.float32))
    ep = np.asarray(inputs["edge_preds"], dtype=np.float32)

    # argmax over the 2 class logits: class 1 iff ep[1] > ep[0] (ties -> 0)
    edge = ep[:, 1, :] > ep[:, 0, :]
    any_e = edge.any(axis=1)
    all_e = edge.all(axis=1)
    fast = any_e & ~all_e  # GCN layers are exact identities

    out = _run_device(seg)  # 2 * transpose, correct wherever fast[b]

    if not fast.all():
        g_all = np.asarray(inputs["gmatrix"], dtype=np.float32)
        W1 = np.asarray(inputs["W1"], dtype=np.float32)
        b1 = np.asarray(inputs["b1"], dtype=np.float32)
        W2 = np.asarray(inputs["W2"], dtype=np.float32)
        b2 = np.asarray(inputs["b2"], dtype=np.float32)
        for b in range(B):
            if not fast[b]:
                feat = np.ascontiguousarray(seg[b].T)
                r = _np_sample(feat, ep[b], g_all[b], W1, b1, W2, b2)
                out[b] = r + feat
    return out


# revision 7
# speedup vs baseline: 1.0957x; 1.0363x over previous
"""Trainium2 kernel for nn_BFM_torch_56384330662315 (gnn_message_passing).

Reference semantics (B=4, C=128, N=2048, K=16):
  feats = transpose(seg_features, (0,2,1))                 # [B,N,C]
  per sample: adj = boundary-cut symmetric kNN graph; two GCN layers
  out = refined + feats

Each GCN layer computes ``out = (adj/deg) @ f + feat`` but returns plain
``feat`` whenever any node has zero degree (``has_zero`` in the reference).
Any node classified as a boundary node (argmax(edge_preds)==1) gets its row
AND column zeroed in the symmetric adjacency, so it has zero degree.  Hence
for every sample that has at least one edge node and at least one non-edge
node, both GCN layers are exact identities and the whole module reduces,
bit-for-bit in fp32, to:

  out = 2 * transpose(seg_features, (0,2,1))

The device kernel evaluates that scaled transpose, data-parallel over the
8 NeuronCores.  Each core handles a [128, 1024] half-sample in bfloat16
(rel err ~3e-3, well under the 2e-2 gate), raw bass with manual
semaphores (TileContext's epilogue barrier costs ~500ns of tail):

  - two DMA-transpose (XBAR) loads bring x^T into SBUF already transposed
    ([4,4] blocks: the second transfer starts right as the first ends,
    given HWDGE descriptor generations serialize at ~625ns);
  - DVE applies the x2 in two chunks (the only arithmetic left);
  - two SP HWDGE stores write the result via 2KB/partition descriptors
    into a packed [128, 8*128] DRAM layout (the host unpermutes, a pure
    reshape), their descriptor generations pipelined behind the scales.

(Prepared/triggered SWDGE stores would cut another ~1.2us of store-issue
latency and sim fine, but crash real silicon - NRT_EXEC_UNIT_UNRECOVERABLE;
the deferred-read ordering the simulators honor is not enforced by the
hardware.  Confirmed broken in two separate sessions; do not retry.)

The per-sample condition is checked on host from edge_preds (tiny);
samples that don't satisfy it (probability ~2^-2047 for the randn inputs
this problem is generated with) fall back to an exact numpy port of the
reference.
"""

import os

import numpy as np

# recover cleanly if a previous kernel left a NeuronCore exec unit wedged
os.environ.setdefault("NEURON_RT_RESET_CORES", "1")

B, C, N, K = 4, 128, 2048, 16
GEO_FILL = 1000.0
NCORES = 8
NSH = N * B // NCORES  # 1024 columns of seg_features per core
NBLK = NSH // 128      # 8 transpose blocks per core

# per-core program shape (sweep-tuned in TimelineSim)
SPLIT = [4, 4]

_EXEC = None  # cached (jitted shard_map, metadata)


def _bf16():
    import ml_dtypes

    return np.dtype(ml_dtypes.bfloat16)


def _build_nc():
    """Per-core program: y[128, 8*128] = packed 2*x[128,1024]^T in bf16.

    y[p, i*128 + c] = 2 * x[c, i*128 + p]  (host unpermutes blocks).

    Raw bass (no TileContext — its epilogue barrier costs ~500ns on the
    tail); manual semaphores order the three stages.  Pipeline
    (TimelineSim-tuned, [4,4] block splits): two XBAR dma-transpose loads
    (HWDGE descriptor generations serialize at ~625ns, so the second
    transfer starts right as the first ends), two DVE x2 scales (DVE is 3x
    faster than ACT here), two SP HWDGE stores whose descriptor
    generations pipeline exactly behind the scales.
    """
    from concourse import bacc, mybir

    BF16 = mybir.dt.bfloat16
    nc = bacc.Bacc(
        "TRN2",
        target_bir_lowering=False,
        debug=False,
        num_devices=NCORES,
    )
    # Trim dead framework preamble (~590ns of the critical path):
    #  - the 4 const-tile memsets serialize on Pool and gate the start
    #    barrier, but nothing in this program reads the const tiles; move
    #    them after the barrier point so they run in Pool's idle window;
    #  - the all-engine start barrier itself (the preamble EventSemaphore
    #    handshake) protects pre-barrier init that no longer exists; every
    #    cross-engine ordering in this program is expressed with explicit
    #    semaphores, so drop it.  Engine-local setup (Drains, register
    #    moves, TPB base loads) stays, in program order, on each engine.
    blk = nc.m.functions[0].blocks[0]
    insts = list(blk.instructions)
    memsets = [i for i in insts if isinstance(i, mybir.InstMemset)]
    assert len(memsets) == 4, [str(i) for i in memsets]
    kept = [
        i
        for i in insts
        if not isinstance(i, mybir.InstMemset)
        and not isinstance(i, mybir.InstEventSemaphore)
    ]
    blk.instructions = kept + memsets

    x = nc.dram_tensor("x", [C, NSH], BF16, kind="ExternalInput").ap()
    y = nc.dram_tensor("y", [128, NSH], BF16, kind="ExternalOutput").ap()

    xt = nc.alloc_sbuf_tensor("xt_raw", [128, NBLK, 128], BF16).ap()
    ot = nc.alloc_sbuf_tensor("ot_raw", [128, NBLK, 128], BF16).ap()
    x_sem = nc.alloc_semaphore("x_sem")
    s_sem = nc.alloc_semaphore("s_sem")
    y_sem = nc.alloc_semaphore("y_sem")

    b0 = 0
    for nb in SPLIT:
        nc.sync.dma_start(
            xt[:, b0 : b0 + nb, :],
            x[:, b0 * 128 : (b0 + nb) * 128],
            transpose=True,
        ).then_inc(x_sem, 16)
        b0 += nb

    b0 = 0
    for j, nb in enumerate(SPLIT):
        nc.vector.wait_ge(x_sem, 16 * (j + 1))
        nc.vector.tensor_scalar_mul(
            ot[:, b0 : b0 + nb, :], xt[:, b0 : b0 + nb, :], 2.0
        ).then_inc(s_sem, 1)
        b0 += nb

    b0 = 0
    for j, nb in enumerate(SPLIT):
        nc.sync.wait_ge(s_sem, j + 1)
        nc.sync.dma_start(
            y[:, b0 * 128 : (b0 + nb) * 128],
            ot[:, b0 : b0 + nb, :],
        ).then_inc(y_sem, 16)
        b0 += nb

    nc.sync.wait_ge(y_sem, 16 * len(SPLIT))
    nc.compile()
    return nc


def _get_exec():
    """Build the per-core Bass program once and wrap it in a cached
    jit(shard_map) over the 8 cores (mirrors bass2jax.run_bass_via_pjrt)."""
    global _EXEC
    if _EXEC is not None:
        return _EXEC

    import jax
    from jax.experimental.shard_map import shard_map
    from jax.sharding import Mesh, PartitionSpec

    from concourse import bass2jax, mybir

    bass2jax.install_neuronx_cc_hook()
    nc = _build_nc()
    partition_name = nc.partition_id_tensor.name if nc.partition_id_tensor else None

    in_names: list = []
    out_names: list = []
    out_avals: list = []
    zero_outs: list = []
    for alloc in nc.m.functions[0].allocations:
        if not isinstance(alloc, mybir.MemoryLocationSet):
            continue
        name = alloc.memorylocations[0].name
        if alloc.kind == "ExternalInput":
            if name != partition_name:
                in_names.append(name)
        elif alloc.kind == "ExternalOutput":
            out_names.append(name)
            shape = tuple(alloc.tensor_shape)
            dtype = mybir.dt.np(alloc.dtype)
            out_avals.append(jax.core.ShapedArray(shape, dtype))
            zero_outs.append(np.zeros(shape, dtype))
    n_params = len(in_names)
    n_outs = len(out_avals)
    all_names = in_names + out_names
    if partition_name is not None:
        all_names.append(partition_name)

    def _body(*args):
        operands = list(args)
        if partition_name is not None:
            operands.append(bass2jax.partition_id_tensor())
        outs = bass2jax._bass_exec_p.bind(
            *operands,
            out_avals=tuple(out_avals),
            in_names=tuple(all_names),
            out_names=tuple(out_names),
            lowering_input_output_aliases=(),
            sim_require_finite=True,
            sim_require_nnan=True,
            nc=nc,
        )
        return tuple(outs)

    devices = jax.devices()[:NCORES]
    assert len(devices) == NCORES, f"need {NCORES} cores, have {len(jax.devices())}"
    mesh = Mesh(np.asarray(devices), ("core",))
    in_specs = (PartitionSpec("core"),) * (n_params + n_outs)
    out_specs = (PartitionSpec("core"),) * n_outs
    donate = tuple(range(n_params, n_params + n_outs))
    sharded = jax.jit(
        shard_map(
            _body, mesh=mesh, in_specs=in_specs, out_specs=out_specs, check_rep=False
        ),
        donate_argnums=donate,
        keep_unused=True,
    )
    _EXEC = (sharded, in_names, out_names, out_avals, zero_outs)
    return _EXEC


def _make_concat_inputs(seg: np.ndarray):
    """Per-core bf16 input shards, concatenated on axis 0 for shard_map."""
    bf16 = _bf16()
    xs = []
    for k in range(NCORES):
        b, h = k // 2, k % 2
        xs.append(seg[b, :, h * NSH : (h + 1) * NSH].astype(bf16))
    return {"x": np.concatenate(xs, axis=0)}


def _run_device(seg: np.ndarray) -> np.ndarray:
    """seg [B,C,N] f32 -> 2*transpose [B,N,C] f32 on the 8 cores (bf16
    internally, rel err ~1e-3), with retry and a host fallback in case a
    previous session left the accelerator wedged."""
    last_err = None
    for attempt in range(2):
        try:
            return _run_device_once(seg)
        except Exception as e:  # transient NRT_EXEC_UNIT_UNRECOVERABLE etc.
            last_err = e
    import sys

    print(
        f"kernel: device path failed twice ({type(last_err).__name__}: "
        f"{last_err}); computing on host",
        file=sys.stderr,
    )
    return np.ascontiguousarray(2.0 * seg.transpose(0, 2, 1))


def _run_device_once(seg: np.ndarray) -> np.ndarray:
    sharded, in_names, out_names, out_avals, zero_outs = _get_exec()
    by_name = _make_concat_inputs(seg)
    concat_in = [by_name[n] for n in in_names]
    concat_zeros = [
        np.zeros((NCORES * z.shape[0], *z.shape[1:]), z.dtype) for z in zero_outs
    ]
    out_arrs = sharded(*concat_in, *concat_zeros)
    y = np.asarray(out_arrs[out_names.index("y")]).reshape(NCORES, 128, NBLK, 128)

    out = np.empty((B, N, C), dtype=np.float32)
    for k in range(NCORES):
        b, h = k // 2, k % 2
        # y[k][p, i, c] = 2*x[c, i*128+p] -> rows i*128+p of the half-sample
        out[b, h * NSH : (h + 1) * NSH, :] = (
            y[k].transpose(1, 0, 2).reshape(NSH, C).astype(np.float32)
        )
    return out


# ---------------------------------------------------------------------------
# Exact numpy port of the reference — fallback for samples where the GCN does
# not collapse to identity (never hit for this problem's input distribution).
# ---------------------------------------------------------------------------


def _np_build_adj(g, edge_cls, k):
    n = g.shape[0]
    nbrs = np.argsort(g, axis=-1, kind="stable")[:, :k]
    rows = np.arange(n)[:, None]
    adj = np.zeros((n, n), g.dtype)
    adj[rows, nbrs] = 1.0
    adj[nbrs, rows] = 1.0
    is_edge = edge_cls == 1
    adj = np.where(is_edge[:, None], 0.0, adj)
    edge_col = is_edge[None, :]
    cond = (adj == 1) & edge_col
    maxgeo = np.min(np.where(cond, g, GEO_FILL), axis=-1)
    adjr = np.where(g > maxgeo[:, None], 0.0, adj)
    adjr = np.where(edge_col, 0.0, adjr)
    adj2 = np.where(is_edge[:, None], 0.0, adjr)
    adj_sym = ((adj2 > 0) | (adj2.T > 0)).astype(g.dtype)
    if np.all(is_edge):
        return np.eye(n, dtype=g.dtype)
    return adj_sym


def _np_gcn(feat, adj, W, b):
    identity = feat
    f = np.maximum(feat @ W.T + b, 0.0).astype(np.float32)
    row_deg = np.sum(adj, axis=-1, keepdims=True)
    col_deg = np.sum(adj, axis=-2, keepdims=True)
    degree = np.sqrt(row_deg) @ np.sqrt(col_deg)
    if np.any(degree == 0):
        return identity
    out = (adj / degree) @ f + identity
    return out.astype(np.float32)


def _np_sample(feat, ep, g, W1, b1, W2, b2):
    edge_cls = np.argmax(ep, axis=0)
    adj = _np_build_adj(g, edge_cls, K)
    r = _np_gcn(feat, adj, W1, b1)
    r = _np_gcn(r, adj, W2, b2)
    return r


def kernel(**inputs) -> np.ndarray:
    seg = np.ascontiguousarray(np.asarray(inputs["seg_features"], dtype=np.float32))
    ep = np.asarray(inputs["edge_preds"], dtype=np.float32)

    # argmax over the 2 class logits: class 1 iff ep[1] > ep[0] (ties -> 0)
    edge = ep[:, 1, :] > ep[:, 0, :]
    any_e = edge.any(axis=1)
    all_e = edge.all(axis=1)
    fast = any_e & ~all_e  # GCN layers are exact identities

    out = _run_device(seg)  # 2 * transpose, correct wherever fast[b]

    if not fast.all():
        g_all = np.asarray(inputs["gmatrix"], dtype=np.float32)
        W1 = np.asarray(inputs["W1"], dtype=np.float32)
        b1 = np.asarray(inputs["b1"], dtype=np.float32)
        W2 = np.asarray(inputs["W2"], dtype=np.float32)
        b2 = np.asarray(inputs["b2"], dtype=np.float32)
        for b in range(B):
            if not fast[b]:
                feat = np.ascontiguousarray(seg[b].T)
                r = _np_sample(feat, ep[b], g_all[b], W1, b1, W2, b2)
                out[b] = r + feat
    return out
